# Initial kernel scaffold
#
"""Causal self-attention (B=2, T=4096, C=768, H=12, D=64) on 8 trn2 cores.

Sharding: (B, H) -> 24 (batch, head) pairs, 3 heads per core.
Core c handles batch b = c // 4 and heads 3*(c%4) .. 3*(c%4)+2.

v2: all matmul operands in bf16 (PSUM accumulation stays fp32), scalar
engine reserved for exp only, fast reciprocal on DVE + gpsimd partition
broadcast for the softmax renorm, one unified 3-head software pipeline
per key-tile pair so the PE never idles (keeps the HAM clock gate at
K=8/8 = 2.4 GHz).

Per-core layouts (all bf16 unless noted):
  xT   [768, 4096]   x[b].T so projections stream tokens in the free dim
  wqk  [768, 3, 128] col groups: [Wq_h0|Wq_h1], [Wk_h0|Wk_h1], [Wq_h2|Wk_h2]
  wvp  [768, 192]    [Wv_h0 Wv_h1 Wv_h2]
  wp   [3, 64, 768]  Wproj row chunk per head
  consts [128, 2144] 4 causal masks [128,512] + vbig ones columns
Heads 0/1 are row-paired on the PE (head 0 in partitions 0-63, head 1 in
64-127) for the D=64-contraction score matmuls; head 2 runs solo in
partitions 0-63. Scores are computed transposed (ST[k, q]) so the PV
matmul contracts k on the partition dim with V in natural [t, d] layout.
Row sums ride an appended ones-column on V (flash-style, no running max).
"""

import numpy as np

B, T, C, H, D = 2, 4096, 768, 12, 64
HPC = 3          # heads per core
NCORES = 8
QB = 512         # query block (psum bank width in fp32)
NQB = T // QB    # 8
KT = 128         # key tile
NKT = T // KT    # 32
VSTRIDE = 200    # per-k-tile column stride in vbig (3*65 used + 5 pad)

_COMPILED = {}
LAST = {}


def _emit(nc, tile, mybir, tc, ctx, aps):
    F32 = mybir.dt.float32
    BF16 = mybir.dt.bfloat16
    EXP = mybir.ActivationFunctionType.Exp
    xT, wqk, wvp, wp, consts, out = aps
    CC = C // 128  # 6 contraction chunks for the projections

    wpool = ctx.enter_context(tc.tile_pool(name="w", bufs=1))
    qkvpool = ctx.enter_context(tc.tile_pool(name="qkv", bufs=1))
    xpool = ctx.enter_context(tc.tile_pool(name="x", bufs=6))
    ptpool = ctx.enter_context(tc.tile_pool(name="pt", bufs=3))
    atpool = ctx.enter_context(tc.tile_pool(name="at", bufs=2))
    opool = ctx.enter_context(tc.tile_pool(name="osb", bufs=3))
    rpool = ctx.enter_context(tc.tile_pool(name="r", bufs=3))
    stp = ctx.enter_context(tc.tile_pool(name="stp", bufs=2, space="PSUM"))
    osp = ctx.enter_context(tc.tile_pool(name="osp", bufs=3, space="PSUM"))
    pjp = ctx.enter_context(tc.tile_pool(name="pjp", bufs=1, space="PSUM"))

    # ---- constants and weights (wqk first: first matmul needs only it + x) ----
    wqk_sb = wpool.tile([128, CC * 3 * 128], BF16)
    nc.sync.dma_start(
        wqk_sb[:].rearrange("p (a g m) -> p a g m", a=CC, g=3),
        wqk.rearrange("(a p) g m -> p a g m", p=128),
    )
    xpre = []
    for tb in range(2):
        for half in range(2):
            xt = xpool.tile([128, 3 * QB], BF16, tag="xt", name=f"xpre{tb}_{half}")
            nc.sync.dma_start(
                xt[:].rearrange("p (a t) -> p a t", a=3),
                xT[
                    384 * half : 384 * (half + 1), tb * QB : (tb + 1) * QB
                ].rearrange("(a p) t -> p a t", p=128),
            )
            xpre.append(xt)
    wvp_sb = wpool.tile([128, CC * 192], BF16)
    nc.sync.dma_start(
        wvp_sb[:].rearrange("p (a n) -> p a n", a=CC),
        wvp.rearrange("(a p) n -> p a n", p=128),
    )
    masks_sb = wpool.tile([128, 2048], BF16)
    nc.sync.dma_start(masks_sb[:], consts[:, 0:2048])
    wp_sb = wpool.tile([64, 3 * C], BF16)
    nc.sync.dma_start(
        wp_sb[:].rearrange("p (g n) -> p g n", g=3),
        wp.rearrange("g p n -> p g n"),
    )

    # ---- qkv storage ----
    # qkT01: [0:T] = qT (h0 rows 0-63, h1 rows 64-127), [T:2T] = kT
    qkT01 = qkvpool.tile([128, 2 * T], BF16)
    # qk2: rows 0-63 only: [0:T] = qT_h2, [T:2T] = kT_h2
    qk2 = qkvpool.tile([64, 2 * T], BF16)
    vbig = qkvpool.tile([128, NKT * VSTRIDE], BF16)
    vbig3 = vbig[:].rearrange("p (t c) -> p t c", c=VSTRIDE)
    # ones columns of vbig (col 65h+64 per k-tile): gpsimd memset, not DMA
    # (a 2-byte-element scatter DMA takes ~9us and blocks the x prefetch)
    for h in range(3):
        nc.gpsimd.memset(vbig3[:, :, 65 * h + 64 : 65 * h + 65], 1.0)

    # ---- phase 1: qkv projections ----
    for tb in range(NQB):
        t0 = tb * QB
        if tb < 2:
            xh = xpre[2 * tb : 2 * tb + 2]
        else:
            xh = []
            for half in range(2):
                xt = xpool.tile([128, 3 * QB], BF16, tag="xt")
                nc.sync.dma_start(
                    xt[:].rearrange("p (a t) -> p a t", a=3),
                    xT[384 * half : 384 * (half + 1), t0 : t0 + QB].rearrange(
                        "(a p) t -> p a t", p=128
                    ),
                )
                xh.append(xt)

        def xchunk(cc):
            return xh[cc // 3][:, (cc % 3) * QB : (cc % 3 + 1) * QB]

        # pair q then pair k into one [128, 1024] psum tile
        ps_qk = stp.tile([128, 2 * QB], F32, tag="st")
        for g in range(2):
            for cc in range(CC):
                nc.tensor.matmul(
                    ps_qk[:, g * QB : (g + 1) * QB],
                    wqk_sb[:, (cc * 3 + g) * 128 : (cc * 3 + g + 1) * 128],
                    xchunk(cc),
                    start=(cc == 0),
                    stop=(cc == CC - 1),
                )
        nc.vector.tensor_copy(qkT01[:, t0 : t0 + QB], ps_qk[:, 0:QB])
        nc.vector.tensor_copy(qkT01[:, T + t0 : T + t0 + QB], ps_qk[:, QB : 2 * QB])
        # head 2 q and k separately (M=64, rows 0-63)
        ps_h2 = stp.tile([128, 2 * QB], F32, tag="st")
        for g2 in range(2):
            for cc in range(CC):
                base = (cc * 3 + 2) * 128 + 64 * g2
                nc.tensor.matmul(
                    ps_h2[0:64, g2 * QB : (g2 + 1) * QB],
                    wqk_sb[:, base : base + 64],
                    xchunk(cc),
                    start=(cc == 0),
                    stop=(cc == CC - 1),
                )
            nc.vector.tensor_copy(
                qk2[:, g2 * T + t0 : g2 * T + t0 + QB],
                ps_h2[0:64, g2 * QB : (g2 + 1) * QB],
            )
        # v: natural [t, d] layout, 4 k-tiles per tb
        for tt in range(4):
            kt = 4 * tb + tt
            ps = pjp.tile([128, QB], F32, tag="m")
            for cc in range(CC):
                nc.tensor.matmul(
                    ps[:, 0:192],
                    xchunk(cc)[:, tt * 128 : (tt + 1) * 128],
                    wvp_sb[:, cc * 192 : (cc + 1) * 192],
                    start=(cc == 0),
                    stop=(cc == CC - 1),
                )
            dst = vbig3[:, kt, 0:195].rearrange("p (h c) -> p h c", c=65)[:, :, 0:64]
            nc.vector.tensor_copy(dst, ps[:, 0:192].rearrange("p (h d) -> p h d", h=3))

    # ---- phase 2: attention + output projection ----
    heads = [
        (0, qkT01[0:64, 0:T], qkT01[0:64, T : 2 * T]),
        (64, qkT01[64:128, 0:T], qkT01[64:128, T : 2 * T]),
        (0, qk2[:, 0:T], qk2[:, T : 2 * T]),
    ]
    for qb in range(NQB - 1, -1, -1):
        t0 = qb * QB
        nkt = 4 * qb + 4
        o_ps = [osp.tile([65, QB], F32, tag="o", name=f"ops{qb}_{h}") for h in range(3)]
        for g in range(nkt // 2):
            for h, (rg, qT_ap, kT_ap) in enumerate(heads):
                st = stp.tile([128, 2 * QB], F32, tag="st", name=f"st{qb}_{g}_{h}")
                for i in range(2):
                    kt = 2 * g + i
                    nc.tensor.matmul(
                        st[:, i * QB : (i + 1) * QB],
                        kT_ap[:, kt * KT : (kt + 1) * KT],
                        qT_ap[:, t0 : t0 + QB],
                        start=True,
                        stop=True,
                        tile_position=(rg, 0),
                    )
                pt = ptpool.tile([128, 2 * QB], BF16, tag="pt")
                nc.scalar.activation(pt[:], st[:], EXP, scale=float(D) ** -0.5)
                dg = g - 2 * qb
                if dg >= 0:
                    nc.vector.tensor_mul(
                        pt[:], pt[:], masks_sb[:, dg * 1024 : (dg + 1) * 1024]
                    )
                for i in range(2):
                    kt = 2 * g + i
                    nc.tensor.matmul(
                        o_ps[h][:],
                        vbig3[:, kt, 65 * h : 65 * h + 65],
                        pt[:, i * QB : (i + 1) * QB],
                        start=(kt == 0),
                        stop=(kt == nkt - 1),
                    )
        att = []
        for h in range(3):
            au = atpool.tile([64, QB], F32, tag=f"au{h}", name=f"au{qb}_{h}")
            nc.vector.tensor_copy(au[:], o_ps[h][0:64, :])
            srow = rpool.tile([1, QB], F32, tag="sr")
            nc.vector.tensor_copy(srow[:], o_ps[h][64:65, :])
            rs = rpool.tile([1, QB], F32, tag="r")
            nc.vector.reciprocal_approx_fast(rs[:], srow[:])
            rbc = atpool.tile([64, QB], F32, tag=f"rbc{h}", name=f"rbc{qb}_{h}")
            nc.gpsimd.partition_broadcast(rbc[:], rs[:])
            an = atpool.tile([64, QB], BF16, tag=f"an{h}", name=f"an{qb}_{h}")
            nc.vector.tensor_mul(an[:], au[:], rbc[:])
            att.append(an)
        # output projection for this query block
        for tt in range(4):
            osb = opool.tile([128, C], F32, tag="osb")
            for j, (c0, cw) in enumerate(((0, 512), (512, 256))):
                pps = pjp.tile([128, QB], F32, tag="m")
                for h in range(3):
                    nc.tensor.matmul(
                        pps[:, 0:cw],
                        att[h][:, tt * 128 : (tt + 1) * 128],
                        wp_sb[:, h * C + c0 : h * C + c0 + cw],
                        start=(h == 0),
                        stop=(h == 2),
                    )
                nc.vector.tensor_copy(osb[:, c0 : c0 + cw], pps[:, 0:cw])
            nc.sync.dma_start(out[t0 + tt * 128 : t0 + (tt + 1) * 128, :], osb[:])


def _build():
    import concourse.bass as bass  # noqa: F401
    import concourse.tile as tile
    import concourse.mybir as mybir
    from concourse import bacc
    from contextlib import ExitStack

    F32 = mybir.dt.float32
    BF16 = mybir.dt.bfloat16
    nc = bacc.Bacc()
    xT = nc.dram_tensor("xT", [C, T], BF16, kind="ExternalInput").ap()
    wqk = nc.dram_tensor("wqk", [C, 3, 128], BF16, kind="ExternalInput").ap()
    wvp = nc.dram_tensor("wvp", [C, 192], BF16, kind="ExternalInput").ap()
    wp = nc.dram_tensor("wp", [3, 64, C], BF16, kind="ExternalInput").ap()
    consts = nc.dram_tensor("consts", [128, 2048], BF16, kind="ExternalInput").ap()
    out = nc.dram_tensor("out", [T, C], F32, kind="ExternalOutput").ap()

    with tile.TileContext(nc) as tc, ExitStack() as ctx:
        _emit(nc, tile, mybir, tc, ctx, (xT, wqk, wvp, wp, consts, out))
    nc.compile()
    return nc


def _consts_np():
    import ml_dtypes

    consts = np.zeros((128, 2048), np.float32)
    p = np.arange(128)[:, None]
    f = np.arange(512)[None, :]
    for m in range(4):
        rel = f - 128 * m
        mask = np.where(rel < 128, np.where(rel >= p, 1.0, 0.0), 1.0)
        mask = np.where(rel < 0, 0.0, mask)
        consts[:, m * 512 : (m + 1) * 512] = mask
    return consts.astype(ml_dtypes.bfloat16)


def _shard_inputs(x, Wqkv, Wproj):
    import ml_dtypes

    BF = ml_dtypes.bfloat16
    consts = _consts_np()
    in_maps = []
    for c in range(NCORES):
        b = c // 4
        hs = [3 * (c % 4) + j for j in range(HPC)]
        wqk = np.zeros((C, 3, 128), np.float32)
        wqk[:, 0, 0:64] = Wqkv[:, (0 * H + hs[0]) * D : (0 * H + hs[0] + 1) * D]
        wqk[:, 0, 64:128] = Wqkv[:, (0 * H + hs[1]) * D : (0 * H + hs[1] + 1) * D]
        wqk[:, 1, 0:64] = Wqkv[:, (1 * H + hs[0]) * D : (1 * H + hs[0] + 1) * D]
        wqk[:, 1, 64:128] = Wqkv[:, (1 * H + hs[1]) * D : (1 * H + hs[1] + 1) * D]
        wqk[:, 2, 0:64] = Wqkv[:, (0 * H + hs[2]) * D : (0 * H + hs[2] + 1) * D]
        wqk[:, 2, 64:128] = Wqkv[:, (1 * H + hs[2]) * D : (1 * H + hs[2] + 1) * D]
        wvp = np.zeros((C, 192), np.float32)
        for j, h in enumerate(hs):
            wvp[:, j * 64 : (j + 1) * 64] = Wqkv[
                :, (2 * H + h) * D : (2 * H + h + 1) * D
            ]
        wp = np.stack([Wproj[h * D : (h + 1) * D, :] for h in hs]).astype(np.float32)
        in_maps.append(
            {
                "xT": np.ascontiguousarray(x[b].T).astype(BF),
                "wqk": wqk.astype(BF),
                "wvp": wvp.astype(BF),
                "wp": wp.astype(BF),
                "consts": consts,
            }
        )
    return in_maps


def kernel(x, Wqkv, Wproj, bproj):
    from concourse.bass_utils import run_bass_kernel_spmd

    x = np.asarray(x, np.float32)
    Wqkv = np.asarray(Wqkv, np.float32)
    Wproj = np.asarray(Wproj, np.float32)
    bproj = np.asarray(bproj, np.float32)

    if "nc" not in _COMPILED:
        _COMPILED["nc"] = _build()
    nc = _COMPILED["nc"]

    in_maps = _shard_inputs(x, Wqkv, Wproj)
    r = run_bass_kernel_spmd(nc, in_maps, list(range(NCORES)))
    LAST["res"] = r
    res = r.results
    out = np.zeros((B, T, C), np.float32)
    for c in range(NCORES):
        out[c // 4] += res[c]["out"]
    out += bproj[None, None, :]
    return out



# revision 1
# speedup vs baseline: 1.0637x; 1.0637x over previous
"""Causal self-attention (B=2, T=4096, C=768, H=12, D=64) on 8 trn2 cores.

Sharding: (B, H) -> 24 (batch, head) pairs, 3 heads per core.
Core c handles batch b = c // 4 and heads 3*(c%4) .. 3*(c%4)+2.

v2: all matmul operands in bf16 (PSUM accumulation stays fp32), scalar
engine reserved for exp only, fast reciprocal on DVE + gpsimd partition
broadcast for the softmax renorm, one unified 3-head software pipeline
per key-tile pair so the PE never idles (keeps the HAM clock gate at
K=8/8 = 2.4 GHz).

Per-core layouts (all bf16 unless noted):
  xT   [768, 4096]   x[b].T so projections stream tokens in the free dim
  wqk  [768, 3, 128] col groups: [Wq_h0|Wq_h1], [Wk_h0|Wk_h1], [Wq_h2|Wk_h2]
  wvp  [768, 192]    [Wv_h0 Wv_h1 Wv_h2]
  wp   [3, 64, 768]  Wproj row chunk per head
  consts [128, 2144] 4 causal masks [128,512] + vbig ones columns
Heads 0/1 are row-paired on the PE (head 0 in partitions 0-63, head 1 in
64-127) for the D=64-contraction score matmuls; head 2 runs solo in
partitions 0-63. Scores are computed transposed (ST[k, q]) so the PV
matmul contracts k on the partition dim with V in natural [t, d] layout.
Row sums ride an appended ones-column on V (flash-style, no running max).
"""

import numpy as np

B, T, C, H, D = 2, 4096, 768, 12, 64
HPC = 3          # heads per core
NCORES = 8
QB = 512         # query block (psum bank width in fp32)
NQB = T // QB    # 8
KT = 128         # key tile
NKT = T // KT    # 32
VSTRIDE = 200    # per-k-tile column stride in vbig (3*65 used + 5 pad)

_COMPILED = {}
LAST = {}


def _emit(nc, tile, mybir, tc, ctx, aps):
    F32 = mybir.dt.float32
    BF16 = mybir.dt.bfloat16
    EXP = mybir.ActivationFunctionType.Exp
    xT, wqk, wvp, wp, consts, out = aps
    CC = C // 128  # 6 contraction chunks for the projections

    wpool = ctx.enter_context(tc.tile_pool(name="w", bufs=1))
    qkvpool = ctx.enter_context(tc.tile_pool(name="qkv", bufs=1))
    xpool = ctx.enter_context(tc.tile_pool(name="x", bufs=6))
    ptpool = ctx.enter_context(tc.tile_pool(name="pt", bufs=3))
    atpool = ctx.enter_context(tc.tile_pool(name="at", bufs=2))
    opool = ctx.enter_context(tc.tile_pool(name="osb", bufs=3))
    rpool = ctx.enter_context(tc.tile_pool(name="r", bufs=3))
    stp = ctx.enter_context(tc.tile_pool(name="stp", bufs=2, space="PSUM"))
    osp = ctx.enter_context(tc.tile_pool(name="osp", bufs=3, space="PSUM"))
    pjp = ctx.enter_context(tc.tile_pool(name="pjp", bufs=1, space="PSUM"))

    # ---- constants and weights (wqk first: first matmul needs only it + x) ----
    wqk_sb = wpool.tile([128, CC * 3 * 128], BF16)
    nc.sync.dma_start(
        wqk_sb[:].rearrange("p (a g m) -> p a g m", a=CC, g=3),
        wqk.rearrange("(a p) g m -> p a g m", p=128),
    )
    xpre = []
    for tb in range(2):
        for half in range(2):
            xt = xpool.tile([128, 3 * QB], BF16, tag="xt", name=f"xpre{tb}_{half}")
            nc.sync.dma_start(
                xt[:].rearrange("p (a t) -> p a t", a=3),
                xT[
                    384 * half : 384 * (half + 1), tb * QB : (tb + 1) * QB
                ].rearrange("(a p) t -> p a t", p=128),
            )
            xpre.append(xt)
    wvp_sb = wpool.tile([128, CC * 192], BF16)
    nc.sync.dma_start(
        wvp_sb[:].rearrange("p (a n) -> p a n", a=CC),
        wvp.rearrange("(a p) n -> p a n", p=128),
    )
    masks_sb = wpool.tile([128, 2048], BF16)
    nc.sync.dma_start(masks_sb[:], consts[:, 0:2048])
    wp_sb = wpool.tile([64, 3 * C], BF16)
    nc.sync.dma_start(
        wp_sb[:].rearrange("p (g n) -> p g n", g=3),
        wp.rearrange("g p n -> p g n"),
    )

    # ---- qkv storage ----
    # qkT01: [0:T] = qT (h0 rows 0-63, h1 rows 64-127), [T:2T] = kT
    qkT01 = qkvpool.tile([128, 2 * T], BF16)
    # qk2: rows 0-63 only: [0:T] = qT_h2, [T:2T] = kT_h2
    qk2 = qkvpool.tile([64, 2 * T], BF16)
    vbig = qkvpool.tile([128, NKT * VSTRIDE], BF16)
    vbig3 = vbig[:].rearrange("p (t c) -> p t c", c=VSTRIDE)
    # ones columns of vbig (col 65h+64 per k-tile): gpsimd memset, not DMA
    # (a 2-byte-element scatter DMA takes ~9us and blocks the x prefetch)
    for h in range(3):
        nc.gpsimd.memset(vbig3[:, :, 65 * h + 64 : 65 * h + 65], 1.0)

    # ---- phase 1: qkv projections ----
    for tb in range(NQB):
        t0 = tb * QB
        if tb < 2:
            xh = xpre[2 * tb : 2 * tb + 2]
        else:
            xh = []
            for half in range(2):
                xt = xpool.tile([128, 3 * QB], BF16, tag="xt")
                nc.sync.dma_start(
                    xt[:].rearrange("p (a t) -> p a t", a=3),
                    xT[384 * half : 384 * (half + 1), t0 : t0 + QB].rearrange(
                        "(a p) t -> p a t", p=128
                    ),
                )
                xh.append(xt)

        def xchunk(cc):
            return xh[cc // 3][:, (cc % 3) * QB : (cc % 3 + 1) * QB]

        # pair q then pair k into one [128, 1024] psum tile
        ps_qk = stp.tile([128, 2 * QB], F32, tag="st")
        for g in range(2):
            for cc in range(CC):
                nc.tensor.matmul(
                    ps_qk[:, g * QB : (g + 1) * QB],
                    wqk_sb[:, (cc * 3 + g) * 128 : (cc * 3 + g + 1) * 128],
                    xchunk(cc),
                    start=(cc == 0),
                    stop=(cc == CC - 1),
                )
        nc.vector.tensor_copy(qkT01[:, t0 : t0 + QB], ps_qk[:, 0:QB])
        nc.vector.tensor_copy(qkT01[:, T + t0 : T + t0 + QB], ps_qk[:, QB : 2 * QB])
        # head 2 q and k separately (M=64, rows 0-63)
        ps_h2 = stp.tile([128, 2 * QB], F32, tag="st")
        for g2 in range(2):
            for cc in range(CC):
                base = (cc * 3 + 2) * 128 + 64 * g2
                nc.tensor.matmul(
                    ps_h2[0:64, g2 * QB : (g2 + 1) * QB],
                    wqk_sb[:, base : base + 64],
                    xchunk(cc),
                    start=(cc == 0),
                    stop=(cc == CC - 1),
                )
            nc.vector.tensor_copy(
                qk2[:, g2 * T + t0 : g2 * T + t0 + QB],
                ps_h2[0:64, g2 * QB : (g2 + 1) * QB],
            )
        # v: natural [t, d] layout, 4 k-tiles per tb
        for tt in range(4):
            kt = 4 * tb + tt
            ps = pjp.tile([128, QB], F32, tag="m")
            for cc in range(CC):
                nc.tensor.matmul(
                    ps[:, 0:192],
                    xchunk(cc)[:, tt * 128 : (tt + 1) * 128],
                    wvp_sb[:, cc * 192 : (cc + 1) * 192],
                    start=(cc == 0),
                    stop=(cc == CC - 1),
                )
            dst = vbig3[:, kt, 0:195].rearrange("p (h c) -> p h c", c=65)[:, :, 0:64]
            nc.vector.tensor_copy(dst, ps[:, 0:192].rearrange("p (h d) -> p h d", h=3))

    # ---- phase 2: attention + output projection ----
    heads = [
        (0, qkT01[0:64, 0:T], qkT01[0:64, T : 2 * T]),
        (64, qkT01[64:128, 0:T], qkT01[64:128, T : 2 * T]),
        (0, qk2[:, 0:T], qk2[:, T : 2 * T]),
    ]
    for qb in range(NQB - 1, -1, -1):
        t0 = qb * QB
        nkt = 4 * qb + 4
        o_ps = [osp.tile([65, QB], F32, tag="o", name=f"ops{qb}_{h}") for h in range(3)]
        for g in range(nkt // 2):
            for h, (rg, qT_ap, kT_ap) in enumerate(heads):
                st = stp.tile([128, 2 * QB], F32, tag="st", name=f"st{qb}_{g}_{h}")
                for i in range(2):
                    kt = 2 * g + i
                    nc.tensor.matmul(
                        st[:, i * QB : (i + 1) * QB],
                        kT_ap[:, kt * KT : (kt + 1) * KT],
                        qT_ap[:, t0 : t0 + QB],
                        start=True,
                        stop=True,
                        tile_position=(rg, 0),
                    )
                pt = ptpool.tile([128, 2 * QB], BF16, tag="pt")
                nc.scalar.activation(pt[:], st[:], EXP, scale=float(D) ** -0.5)
                dg = g - 2 * qb
                if dg >= 0:
                    nc.vector.tensor_mul(
                        pt[:], pt[:], masks_sb[:, dg * 1024 : (dg + 1) * 1024]
                    )
                for i in range(2):
                    kt = 2 * g + i
                    nc.tensor.matmul(
                        o_ps[h][:],
                        vbig3[:, kt, 65 * h : 65 * h + 65],
                        pt[:, i * QB : (i + 1) * QB],
                        start=(kt == 0),
                        stop=(kt == nkt - 1),
                    )
        att = []
        for h in range(3):
            au = atpool.tile([64, QB], F32, tag=f"au{h}", name=f"au{qb}_{h}")
            nc.vector.tensor_copy(au[:], o_ps[h][0:64, :])
            srow = rpool.tile([1, QB], F32, tag="sr")
            nc.vector.tensor_copy(srow[:], o_ps[h][64:65, :])
            rs = rpool.tile([1, QB], F32, tag="r")
            nc.vector.reciprocal_approx_fast(rs[:], srow[:])
            rbc = atpool.tile([64, QB], F32, tag=f"rbc{h}", name=f"rbc{qb}_{h}")
            nc.gpsimd.partition_broadcast(rbc[:], rs[:])
            an = atpool.tile([64, QB], BF16, tag=f"an{h}", name=f"an{qb}_{h}")
            nc.vector.tensor_mul(an[:], au[:], rbc[:])
            att.append(an)
        # output projection for this query block
        for tt in range(4):
            osb = opool.tile([128, C], F32, tag="osb")
            for j, (c0, cw) in enumerate(((0, 512), (512, 256))):
                pps = pjp.tile([128, QB], F32, tag="m")
                for h in range(3):
                    nc.tensor.matmul(
                        pps[:, 0:cw],
                        att[h][:, tt * 128 : (tt + 1) * 128],
                        wp_sb[:, h * C + c0 : h * C + c0 + cw],
                        start=(h == 0),
                        stop=(h == 2),
                    )
                nc.vector.tensor_copy(osb[:, c0 : c0 + cw], pps[:, 0:cw])
            nc.sync.dma_start(out[t0 + tt * 128 : t0 + (tt + 1) * 128, :], osb[:])


def _build():
    import concourse.bass as bass  # noqa: F401
    import concourse.tile as tile
    import concourse.mybir as mybir
    from concourse import bacc
    from contextlib import ExitStack

    F32 = mybir.dt.float32
    BF16 = mybir.dt.bfloat16
    nc = bacc.Bacc()
    xT = nc.dram_tensor("xT", [C, T], BF16, kind="ExternalInput").ap()
    wqk = nc.dram_tensor("wqk", [C, 3, 128], BF16, kind="ExternalInput").ap()
    wvp = nc.dram_tensor("wvp", [C, 192], BF16, kind="ExternalInput").ap()
    wp = nc.dram_tensor("wp", [3, 64, C], BF16, kind="ExternalInput").ap()
    consts = nc.dram_tensor("consts", [128, 2048], BF16, kind="ExternalInput").ap()
    out = nc.dram_tensor("out", [T, C], F32, kind="ExternalOutput").ap()

    with tile.TileContext(nc) as tc, ExitStack() as ctx:
        _emit(nc, tile, mybir, tc, ctx, (xT, wqk, wvp, wp, consts, out))
    nc.compile()
    return nc


def _consts_np():
    import ml_dtypes

    consts = np.zeros((128, 2048), np.float32)
    p = np.arange(128)[:, None]
    f = np.arange(512)[None, :]
    for m in range(4):
        rel = f - 128 * m
        mask = np.where(rel < 128, np.where(rel >= p, 1.0, 0.0), 1.0)
        mask = np.where(rel < 0, 0.0, mask)
        consts[:, m * 512 : (m + 1) * 512] = mask
    return consts.astype(ml_dtypes.bfloat16)


def _shard_inputs(x, Wqkv, Wproj):
    import ml_dtypes

    BF = ml_dtypes.bfloat16
    consts = _consts_np()
    in_maps = []
    for c in range(NCORES):
        b = c // 4
        hs = [3 * (c % 4) + j for j in range(HPC)]
        wqk = np.zeros((C, 3, 128), np.float32)
        wqk[:, 0, 0:64] = Wqkv[:, (0 * H + hs[0]) * D : (0 * H + hs[0] + 1) * D]
        wqk[:, 0, 64:128] = Wqkv[:, (0 * H + hs[1]) * D : (0 * H + hs[1] + 1) * D]
        wqk[:, 1, 0:64] = Wqkv[:, (1 * H + hs[0]) * D : (1 * H + hs[0] + 1) * D]
        wqk[:, 1, 64:128] = Wqkv[:, (1 * H + hs[1]) * D : (1 * H + hs[1] + 1) * D]
        wqk[:, 2, 0:64] = Wqkv[:, (0 * H + hs[2]) * D : (0 * H + hs[2] + 1) * D]
        wqk[:, 2, 64:128] = Wqkv[:, (1 * H + hs[2]) * D : (1 * H + hs[2] + 1) * D]
        wvp = np.zeros((C, 192), np.float32)
        for j, h in enumerate(hs):
            wvp[:, j * 64 : (j + 1) * 64] = Wqkv[
                :, (2 * H + h) * D : (2 * H + h + 1) * D
            ]
        wp = np.stack([Wproj[h * D : (h + 1) * D, :] for h in hs]).astype(np.float32)
        in_maps.append(
            {
                "xT": np.ascontiguousarray(x[b].T).astype(BF),
                "wqk": wqk.astype(BF),
                "wvp": wvp.astype(BF),
                "wp": wp.astype(BF),
                "consts": consts,
            }
        )
    return in_maps


def kernel(x, Wqkv, Wproj, bproj):
    from concourse.bass_utils import run_bass_kernel_spmd

    x = np.asarray(x, np.float32)
    Wqkv = np.asarray(Wqkv, np.float32)
    Wproj = np.asarray(Wproj, np.float32)
    bproj = np.asarray(bproj, np.float32)

    if "nc" not in _COMPILED:
        _COMPILED["nc"] = _build()
    nc = _COMPILED["nc"]

    in_maps = _shard_inputs(x, Wqkv, Wproj)
    r = run_bass_kernel_spmd(nc, in_maps, list(range(NCORES)))
    LAST["res"] = r
    res = r.results
    out = np.zeros((B, T, C), np.float32)
    for c in range(NCORES):
        out[c // 4] += res[c]["out"]
    out += bproj[None, None, :]
    return out

